# revision 10
# baseline (speedup 1.0000x reference)
"""Trainium2 Bass kernel v6 for nn_MemoryCell (B=4096, T=4096), 8 NeuronCores.

Parallel-in-time algorithm restructured around measured TRN2 costs:
- 4 batch groups of 128 rows per core (J=1); group-level software
  pipeline A(g); C(g); D(g) interleaved with A(g+1), so the coarse chain
  and all compute hide under the continuous in/out DMA streams.
- Fine sweep: ONE hw scan per 2048-chunk computes z directly:
      z_t = (base_t * z_{t-1}) * fgate_t,
  fgate = 1 - gp*sigmoid(predictor) piecewise-constant at RP=8, expanded
  once per group by a DVE tensor_scalar reading a broadcast AP (2x perf
  mode; measured scan cost with op1=mult == op1=bypass).
- Coarse fixpoint (1 damped iteration) + predictor per group, all on
  DVE: keeping chains on one engine avoids cross-engine semaphore
  latency (~1us/hop measured), worth more than gpsimd's idle cycles.
- phase-A sigmoid AND base affine both on ACT (same-engine chain).
- 32-block products via DVE tensor_reduce (gpsimd cannot reduce free-axis).
"""

import math
from contextlib import ExitStack

import numpy as np

import concourse.tile as tile
from concourse import bacc, mybir
from concourse.bass_utils import run_bass_kernel_spmd

F32 = mybir.dt.float32
AL = mybir.AluOpType
ACTF = mybir.ActivationFunctionType
AX = mybir.AxisListType

B, T = 4096, 4096
N_CORES = 8
BC = B // N_CORES  # 512 rows per core
P = 128
G = BC // P        # 4 groups per core

LIN = 2048         # phase A chunk length
L = 2048           # phase D chunk length
R = 32             # coarse block length
K = T // R         # 128 coarse blocks
N_COARSE = 1
RP = 8             # predictor piecewise-const segment length
RR = R // RP       # 4 predictor points per coarse block
KP = L // RP       # 256 predictor segments per D chunk
KPF = K * RR       # 512 predictor points per group
NCH = T // LIN     # 2 chunks per group per phase


def _sigmoid(v):
    return 1.0 / (1.0 + math.exp(-v))


def _build(consts, repeat=0, internal_io=False):
    (g_ax, m_ax, s_ax, g_yx, m_yx, s_yx, g_xx, m_xx, s_xx, cap_x, pbar, y0) = consts

    gp = g_xx / cap_x
    c_yx = (g_yx / cap_x) * _sigmoid(s_yx * (y0 - m_yx))
    sg_scale = -s_xx
    sg_bias = s_xx * (pbar - m_xx)
    Aq = -R * gp

    nc = bacc.Bacc("TRN2", target_bir_lowering=False, debug=False)
    kin = "Internal" if internal_io else "ExternalInput"
    kout = "Internal" if internal_io else "ExternalOutput"
    x_in = nc.dram_tensor("x_in", [BC, T, 2], F32, kind=kin).ap()
    y_out = nc.dram_tensor("y_out", [BC, T, 2], F32, kind=kout).ap()
    small = None
    if internal_io:
        small = nc.dram_tensor("small_out", [P, 8], F32, kind="ExternalOutput").ap()
    xd = x_in.rearrange("(g p) t c -> g p t c", p=P)
    yd = y_out.rearrange("(g p) t c -> g p t c", p=P)

    with tile.TileContext(nc) as tc, ExitStack() as ctx:
        pool_c = ctx.enter_context(tc.tile_pool(name="const", bufs=1))
        pool_in = ctx.enter_context(tc.tile_pool(name="pin", bufs=3))
        pool_sa = ctx.enter_context(tc.tile_pool(name="psa", bufs=1))
        pool_base = ctx.enter_context(tc.tile_pool(name="base", bufs=3))
        pool_pr = ctx.enter_context(tc.tile_pool(name="ppr", bufs=2))
        pool_co = ctx.enter_context(tc.tile_pool(name="coarse", bufs=1))
        pool_sgp = ctx.enter_context(tc.tile_pool(name="psgp", bufs=2))
        pool_fg = ctx.enter_context(tc.tile_pool(name="pfg", bufs=2))
        pool_zt = ctx.enter_context(tc.tile_pool(name="pzt", bufs=3))
        pool_out = ctx.enter_context(tc.tile_pool(name="pout", bufs=1))

        state = {}

        def setup():
            cons = pool_c.tile([P, 8], F32, tag="cons")
            nc.vector.memset(cons[:, 0:1], -s_ax * m_ax)
            nc.vector.memset(cons[:, 1:2], sg_bias)
            ramp = pool_c.tile([P, RR], F32, tag="ramp")
            for r in range(RR):
                nc.vector.memset(ramp[:, r : r + 1], (r + 0.5) / RR)
            ochs = [pool_out.tile([P, L, 2], F32, tag=f"och{i}", name=f"och{i}")
                    for i in range(3)]
            for o in ochs:
                nc.gpsimd.memset(o[:, :, 1], y0)
            state.update(cons=cons, ramp=ramp, ochs=ochs, och_i=0)

        def a_chunk(g, c, base, pr):
            bias_sa = state["cons"][:, 0:1]
            t0 = c * LIN
            tin = pool_in.tile([P, LIN, 2], F32, tag="tin", name="tin")
            nc.sync.dma_start(tin[:], xd[g, :, t0 : t0 + LIN, :])
            sa = pool_sa.tile([P, LIN], F32, tag="sa", name="sa")
            nc.scalar.activation(
                sa[:], tin[:, :, 0], ACTF.Sigmoid, bias=bias_sa, scale=s_ax
            )
            bch = base[:, t0 : t0 + LIN]
            nc.scalar.activation(
                bch, sa[:], ACTF.Copy, bias=1.0 - c_yx, scale=-g_ax / cap_x
            )
            nc.vector.tensor_reduce(
                pr[:, c * (LIN // R) : (c + 1) * (LIN // R)],
                bch.rearrange("p (k r) -> p k r", r=R),
                AX.X, AL.mult,
            )

        def bc_group(pr):
            """B (cumprod) + C (coarse fixpoint) + predictor -> sgp, one group."""
            bias_sg = state["cons"][:, 1:2]
            ramp = state["ramp"]
            csamp = pool_co.tile([P, K + 1], F32, tag="csamp")
            nc.vector.memset(csamp[:, 0:1], 1.0)
            nc.vector.tensor_tensor_scan(
                csamp[:, 1 : K + 1], pr[:], pr[:], 1.0, AL.mult, AL.bypass
            )
            # with pbar == 1, zb0 == csamp: read csamp directly in the update
            zb = pool_co.tile([P, K + 1], F32, tag="zb")
            zb0 = csamp if pbar == 1.0 else zb
            if pbar != 1.0:
                nc.vector.tensor_scalar(zb[:], csamp[:], pbar, None, AL.mult)
            zm = pool_co.tile([P, K], F32, tag="zm")
            sgc = pool_co.tile([P, K], F32, tag="sgc")
            qc = pool_co.tile([P, K], F32, tag="qc")
            ec = pool_co.tile([P, K], F32, tag="ec")
            fc = pool_co.tile([P, K], F32, tag="fc")
            nc.vector.tensor_add(zm[:], zb0[:, 0:K], zb0[:, 1 : K + 1])
            nc.scalar.activation(
                sgc[:], zm[:], ACTF.Sigmoid, bias=bias_sg, scale=sg_scale / 2.0
            )
            # exp(q) ~= 1 + q*(1 + q/2), q = Aq*sg  (|q| <= ~0.04;
            # the Bq*sg^2 refinement of q is ~5e-4 cumulative: dropped)
            nc.vector.tensor_scalar(qc[:], sgc[:], Aq * 0.5, 1.0, AL.mult, AL.add)
            nc.vector.scalar_tensor_tensor(ec[:], sgc[:], Aq, qc[:], AL.mult, AL.mult)
            nc.vector.tensor_scalar(ec[:], ec[:], 1.0, None, AL.add)
            nc.vector.tensor_tensor_scan(
                fc[:], ec[:], ec[:], 1.0, AL.mult, AL.bypass
            )
            nc.vector.tensor_mul(zb[:, 1 : K + 1], csamp[:, 1 : K + 1], fc[:])
            if pbar != 1.0:
                nc.vector.tensor_scalar(
                    zb[:, 1 : K + 1], zb[:, 1 : K + 1], pbar, None, AL.mult
                )
            nc.vector.memset(zb[:, 0:1], pbar)
            dzb = pool_co.tile([P, K], F32, tag="dzb")
            nc.vector.tensor_sub(dzb[:], zb[:, 1 : K + 1], zb[:, 0:K])
            zp = pool_co.tile([P, K, RR], F32, tag="zp")
            nc.vector.tensor_mul(
                zp[:],
                dzb[:].unsqueeze(2).broadcast_to([P, K, RR]),
                ramp[:].unsqueeze(1).broadcast_to([P, K, RR]),
            )
            nc.vector.tensor_add(
                zp[:], zp[:], zb[:, 0:K].unsqueeze(2).broadcast_to([P, K, RR])
            )
            sgp = pool_sgp.tile([P, KPF], F32, tag="sgp")
            nc.scalar.activation(
                sgp[:], zp[:].rearrange("p k r -> p (k r)"),
                ACTF.Sigmoid, bias=bias_sg, scale=sg_scale,
            )
            return sgp

        def d_chunk(g, c, base, sgp, zt_prev):
            t0 = c * L
            fg = pool_fg.tile([P, KP, RP], F32, tag="fg", name="fg")
            nc.vector.tensor_scalar(
                fg[:],
                sgp[:, c * KP : (c + 1) * KP]
                .unsqueeze(2).broadcast_to([P, KP, RP]),
                -gp, 1.0, AL.mult, AL.add,
            )
            zt = pool_zt.tile([P, L], F32, tag="zt", name="zt")
            init = pbar if c == 0 else zt_prev[:, L - 1 : L]
            nc.vector.tensor_tensor_scan(
                zt[:], base[:, t0 : t0 + L],
                fg[:].rearrange("p k r -> p (k r)"),
                init, AL.mult, AL.mult,
            )
            och = state["ochs"][state["och_i"] % 3]
            state["och_i"] += 1
            nc.scalar.activation(
                och[:, :, 0], zt[:], ACTF.Copy, bias=float(pbar), scale=-1.0
            )
            nc.scalar.dma_start(yd[g, :, t0 : t0 + L, :], och[:])
            return zt

        def new_base():
            return pool_base.tile([P, T], F32, tag="base", name="base")

        def prog():
            # group-level pipeline: A(g) ; C(g) ; D(g) interleaved with A(g+1)
            prs = {}
            bases = {}
            sgps = {}

            def a_group_start(g):
                prs[g] = pool_pr.tile([P, K], F32, tag="pr", name="pr")
                bases[g] = pool_base.tile([P, T], F32, tag="base", name="base")

            a_group_start(0)
            for c in range(NCH):
                a_chunk(0, c, bases[0], prs[0])
            sgps[0] = bc_group(prs[0])
            for g in range(1, G + 1):
                if g < G:
                    a_group_start(g)
                ztp = None
                for c in range(NCH):
                    ztp = d_chunk(g - 1, c, bases[g - 1], sgps[g - 1], ztp)
                    if g < G:
                        a_chunk(g, c, bases[g], prs[g])
                if g < G:
                    sgps[g] = bc_group(prs[g])

        setup()
        if repeat > 0:
            with tc.For_i(0, repeat, 1) as _i:
                prog()
        else:
            prog()
        if internal_io:
            fin = pool_c.tile([P, 8], F32, tag="fin")
            nc.vector.memset(fin[:], 1.0)
            nc.sync.dma_start(small, fin[:])

    nc.compile()
    return nc


_CACHE = {}


def kernel(inputs: np.ndarray, params: np.ndarray) -> np.ndarray:
    p = np.asarray(params, np.float64)
    cap_x, cap_y = float(p[0]), float(p[1])
    d = p[2:].reshape(6, 4)  # rows: ax, by, xy, yx, xx, yy  (g, mean, std, pot)
    (g_ax, m_ax, s_ax, p_ax) = d[0]
    (g_yx, m_yx, s_yx, p_yx) = d[3]
    (g_xx, m_xx, s_xx, p_xx) = d[4]
    y0 = 1.0  # initial states fixed by the reference: x0=0, y0=1

    y_const = d[1][3] == y0 and d[2][3] == y0 and d[5][3] == y0
    pots_eq = p_ax == p_yx == p_xx
    small = (abs(g_ax) + abs(g_yx) + abs(g_xx)) / abs(cap_x) < 0.05
    if not (y_const and pots_eq and small):
        raise NotImplementedError("general-path params not supported")
    pbar = float(p_ax)

    consts = (
        float(g_ax), float(m_ax), float(s_ax),
        float(g_yx), float(m_yx), float(s_yx),
        float(g_xx), float(m_xx), float(s_xx),
        cap_x, pbar, y0,
    )
    if consts not in _CACHE:
        _CACHE[consts] = _build(consts)
    nc = _CACHE[consts]

    x = np.ascontiguousarray(np.asarray(inputs, np.float32))
    in_maps = [{"x_in": x[c * BC : (c + 1) * BC]} for c in range(N_CORES)]
    res = run_bass_kernel_spmd(nc, in_maps, core_ids=list(range(N_CORES)))
    return np.concatenate([res.results[c]["y_out"] for c in range(N_CORES)], axis=0)
